# revision 51
# baseline (speedup 1.0000x reference)
"""Trainium2 Bass kernel for nn_HFGA_54606214201918.

Computation (per batch element b, C=256 channels, L=4096 positions):
    xh  = (x[:, 0::2] - x[:, 1::2]) / sqrt(2)          # Haar high band  [C, L/2]
    q   = Wq @ x + bq                                  # [C, L]
    k   = Wk @ xh + bk                                 # [C, L/2]
    v   = Wv @ xh + bv                                 # [C, L/2]
    attn = softmax_over_keys((k^T q) / sqrt(C))        # [L/2, L]
    out = (v @ attn) * tanh(gate) + x

Sharding: data-parallel over batch B=8 across the 8 NeuronCores (one batch
element per core); weights broadcast. No collectives.

Key optimizations vs the bf16 baseline:
  - Q projection folded away on host: scores S = xh^T H x with
    H = Wk^T Wq * invsqrt2/sqrt(C).  bk never affects the output (per-query
    shift, softmax-invariant); bq only adds a per-KEY shift u[m] which is
    folded into the exp's per-partition bias (separately compiled variant,
    only used when bq != 0).
  - All heavy matmuls run in fp8 DoubleRow perf mode: one instruction
    contracts TWO K=128 tiles at 0.5 cycles per output column (4x bf16).
    Scores use e4m3 operands (x, k' prescaled to ~unit std on host / at the
    PSUM drain); exp outputs and the v/ones operands use e5m2 (no overflow:
    fp8 casts do NOT saturate on TRN2, they go to inf).
  - Softmax shift: E = exp(S - 3) (softmax-invariant, keeps E in e5m2
    comfortable range); denominator Z via a ones-pair DoubleRow matmul;
    normalization applied to the small [C, LB] output via
    reciprocal_approx_fast + a K=1 broadcast matmul, fused with the
    residual add on the vector + gpsimd engines.
  - exp drains TWO PSUM banks per scalar-engine instruction ([128, 2, 512])
    to halve per-instruction overhead; the scalar engine is the bottleneck.
"""
import sys

if '/opt/trn_rl_repo' not in sys.path:
    sys.path.insert(0, '/opt/trn_rl_repo')

import numpy as np
import ml_dtypes

import concourse.bass as bass
import concourse.tile as tile
from concourse import bacc, mybir
from concourse import bass_utils

B, C, L = 8, 256, 4096
M = L // 2            # 2048 keys
P = 128               # partitions
CO = C // P           # 2 channel chunks
LB = 512              # l-tile (one PSUM bank of fp32)
NB = L // LB          # 8 l-tiles
MJ = M // P           # 16 key chunks
NPAIR = MJ // 2       # 8 key-chunk pairs
INV_SQRT2 = 0.7071067811865476

F32 = mybir.dt.float32
F32R = mybir.dt.float32r
BF16 = mybir.dt.bfloat16
E4 = mybir.dt.float8e4
E5 = mybir.dt.float8e5
AF = mybir.ActivationFunctionType
DR = mybir.MatmulPerfMode.DoubleRow

ALPHA_K = 16.0        # stored k' = ALPHA_K * k'_true (so scores PSUM = 16*S)
BETA_H = 256.0        # host prescale of H so fp8 weights are ~unit std
BETA_V = 32.0         # host prescale of Wv^T
CSH = 3.0             # softmax shift: E = exp(S - CSH)

NE4 = ml_dtypes.float8_e4m3   # matches TRN FP8_EXP4 (max 240, has inf)
NBF = ml_dtypes.bfloat16

_CACHE = {}


def _build(with_ubias=False, with_bv=False):
    nc = bacc.Bacc("TRN2", target_bir_lowering=False, debug=False, num_devices=8)

    xb_d = nc.dram_tensor("xb", [P, CO, L], BF16, kind="ExternalInput").ap()
    x8_d = nc.dram_tensor("x8", [P, CO, L], E4, kind="ExternalInput").ap()
    xh8_d = nc.dram_tensor("xh8", [P, CO, M], E4, kind="ExternalInput").ap()
    wh8_d = nc.dram_tensor("wh8", [P, CO, C], E4, kind="ExternalInput").ap()
    wv8_d = nc.dram_tensor("wv8", [P, CO, C], E4, kind="ExternalInput").ap()
    bvb_d = nc.dram_tensor("bvb", [1, C], BF16, kind="ExternalInput").ap() \
        if with_bv else None
    if with_ubias:
        ub_d = nc.dram_tensor("ub", [P, MJ], F32, kind="ExternalInput").ap()
    y_d = nc.dram_tensor("y", [P, CO, L], BF16, kind="ExternalOutput").ap()

    with tile.TileContext(nc) as tc:
        with tc.tile_pool(name="consts", bufs=1) as consts, \
             tc.tile_pool(name="big", bufs=1) as big, \
             tc.tile_pool(name="e", bufs=8) as e_pool, \
             tc.tile_pool(name="rzp", bufs=2) as rz_pool, \
             tc.tile_pool(name="tp", bufs=2) as t_pool, \
             tc.tile_pool(name="op", bufs=2) as out_pool, \
             tc.tile_pool(name="pssp", bufs=2, space="PSUM") as ps_sp, \
             tc.tile_pool(name="psyh", bufs=1, space="PSUM") as ps_yh, \
             tc.tile_pool(name="psz", bufs=1, space="PSUM") as ps_z, \
             tc.tile_pool(name="psbp", bufs=1, space="PSUM") as ps_bp:

            # ---- constants ----
            wh8 = consts.tile([P, CO, C], E4)
            wv8 = consts.tile([P, CO, C], E4)
            bvb = consts.tile([1, C], BF16, name="bvb") if with_bv else None
            ones_row_bf = consts.tile([1, P], BF16)
            ones_pair_e5 = consts.tile([P, CO, 32], E5)
            ones_row_fr = consts.tile([1, P], F32)
            negc = consts.tile([P, 1], F32)

            # ---- big persistent tensors ----
            x8 = big.tile([P, CO, L], E4)          # scores rhs
            xh8 = big.tile([P, CO, M], E4)         # haar band (raw, no 1/sqrt2)
            xb = big.tile([P, CO, L], BF16)        # residual
            k8 = big.tile([P, CO, M], E4)          # 16 * k'_true, [c, m]
            vt8 = big.tile([P, MJ, C], E5)         # gate*(v+bv), [m, c] chunks
            if with_ubias:
                ub = consts.tile([P, MJ], F32)

            # ---- DMA loads: critical-path tensors first, spread over three
            # queues.  xh8 (k'/v projections) and x8 (scores rhs) gate the
            # pipeline start; xb (residual, bf16) is only needed by the
            # epilogues and trickles in on the sync/vector queues.
            nc.gpsimd.dma_start(out=x8[:, :, :L // 4], in_=x8_d[:, :, :L // 4])
            nc.sync.dma_start(out=wh8, in_=wh8_d)
            nc.sync.dma_start(out=xh8[:, :, :M // 2], in_=xh8_d[:, :, :M // 2])
            nc.gpsimd.dma_start(out=xh8[:, :, M // 2:], in_=xh8_d[:, :, M // 2:])
            nc.sync.dma_start(out=wv8, in_=wv8_d)
            if with_bv:
                nc.sync.dma_start(out=bvb, in_=bvb_d)
            if with_ubias:
                nc.sync.dma_start(out=ub, in_=ub_d)
            for j in range(1, 4):
                sl = slice(j * (L // 4), (j + 1) * (L // 4))
                nc.gpsimd.dma_start(out=x8[:, :, sl], in_=x8_d[:, :, sl])
            for j in range(4):
                sl = slice(j * (L // 4), (j + 1) * (L // 4))
                nc.sync.dma_start(out=xb[:, :, sl], in_=xb_d[:, :, sl])

            nc.vector.memset(ones_row_bf, 1.0)
            nc.vector.memset(ones_pair_e5, 1.0)
            nc.vector.memset(ones_row_fr, 1.0)
            nc.vector.memset(negc, -CSH)

            # PE p-state warm-up: the tensor engine runs at half clock until
            # it has been busy ~3us.  Burn that ramp on dummy broadcasts into
            # the (otherwise still unused) bp bank while the input DMAs are
            # in flight, so the projections and first scores run at full
            # speed.
            warm = consts.tile([1, LB], BF16)
            nc.vector.memset(warm, 0.0)
            for w in range(14):
                wp = ps_bp.tile([P, LB], F32, tag="bp", name=f"warm{w}")
                nc.tensor.matmul(wp, ones_row_bf, warm, start=True, stop=True)

            # ---- projections, k'/v interleaved; k' drains on the (otherwise
            # idle) scalar engine, v drains on the vector engine, so the two
            # chase the PE in parallel instead of serializing on one engine.
            def emit_kproj(b_):
                msl = slice(b_ * LB, (b_ + 1) * LB)
                kp = ps_sp.tile([P, CO, LB], F32, tag="sp", name=f"kp{b_}")
                for oc in range(CO):
                    nc.tensor.matmul(
                        kp[:, oc, :], wh8[:, :, oc * P:(oc + 1) * P],
                        xh8[:, :, msl], start=True, stop=True, perf_mode=DR)
                nc.vector.tensor_scalar_mul(k8[:, :, msl], kp, 1.0 / ALPHA_K)

            def emit_vproj(j):
                vp = ps_sp.tile([P, 2, LB], F32, tag="sp", name=f"vp{j}")
                for i in range(2):
                    mj = 2 * j + i
                    nc.tensor.matmul(
                        vp[:, i, :C], xh8[:, :, mj * P:(mj + 1) * P], wv8,
                        start=True, stop=not with_bv, perf_mode=DR)
                    if with_bv:
                        nc.tensor.matmul(vp[:, i, :C], ones_row_bf, bvb,
                                         start=False, stop=True)
                nc.vector.tensor_scalar_mul(vt8[:, 2 * j:2 * j + 2, :],
                                            vp[:, :, :C], 1.0 / BETA_V)

            # k' bank 0 up front (gates the first scores); the rest of the
            # projections are emitted just-in-time between the first score
            # steps of the global pipeline below.
            emit_kproj(0)
            proj_sched = {
                0: [('k', 1)], 1: [('k', 2)], 2: [('k', 3)],
                3: [('v', 0), ('v', 1)], 4: [('v', 2), ('v', 3)],
                5: [('v', 4), ('v', 5)], 6: [('v', 6), ('v', 7)],
            }

            # ---- attention: ONE global software pipeline over all 64
            # (l-tile, key-pair) steps.  Scores for step g are emitted LAG
            # steps ahead of that step's consumers (yh/z matmuls), ACROSS
            # l-tile boundaries, so the scalar engine (the bottleneck) always
            # has buffered exp work even while an l-tile's epilogue drains.
            # Each epilogue is split: phase 1 (vector ops; the yhp->SBUF copy
            # releases the single yh PSUM buffer fast) runs right after the
            # tile's last consumer; phase 2 (the PE broadcast matmul + the
            # normalize/residual) is deferred one step so the PE queue head
            # never waits on the reciprocal.
            LAG = 5
            TOT = NB * NPAIR
            pend = {}
            state = {}
            pend_ph2 = None
            pend_ph2_g = -1

            def make_phase2(lt, u, rzb, yhp_done):
                sl = slice(lt * LB, (lt + 1) * LB)

                def ph2():
                    bp = ps_bp.tile([P, LB], F32, tag="bp", name=f"bp{lt}")
                    nc.tensor.matmul(bp, ones_row_bf, rzb,
                                     start=True, stop=True)
                    # DVE cannot read two PSUM operands in one op: stage the
                    # broadcast row through SBUF.
                    b_sb = rz_pool.tile([P, LB], F32, tag="bsb",
                                        name=f"bsb{lt}")
                    nc.vector.tensor_copy(b_sb, bp)
                    t = t_pool.tile([P, CO, LB], BF16, tag="t2",
                                    name=f"t{lt}")
                    o = out_pool.tile([P, CO, LB], BF16, tag="o",
                                      name=f"o{lt}")
                    # per-oc, alternating engines, store halves as they land:
                    # shortens the serial chain that forms the kernel tail
                    # after the last exp.
                    for oc in range(CO):
                        nc.vector.tensor_mul(t[:, oc, :], u[:, oc, :], b_sb)
                        (nc.vector if oc else nc.gpsimd).tensor_add(
                            o[:, oc, :], t[:, oc, :], xb[:, oc, sl])
                        nc.gpsimd.dma_start(out=y_d[:, oc, sl],
                                            in_=o[:, oc, :])
                return ph2

            for g in range(TOT + LAG):
                for kind, idx in proj_sched.pop(g, ()):
                    (emit_kproj if kind == 'k' else emit_vproj)(idx)
                if g < TOT:
                    lt, j = divmod(g, NPAIR)
                    sl = slice(lt * LB, (lt + 1) * LB)
                    sp = ps_sp.tile([P, 2, LB], F32, tag="sp",
                                    name=f"sp{lt}_{j}")
                    for i in range(2):
                        mj = 2 * j + i
                        nc.tensor.matmul(
                            sp[:, i, :], k8[:, :, mj * P:(mj + 1) * P],
                            x8[:, :, sl], start=True, stop=True, perf_mode=DR)
                    e = e_pool.tile([P, 2, LB], E5, tag="e", name=f"e{lt}_{j}")
                    if with_ubias:
                        for i in range(2):
                            mj = 2 * j + i
                            nc.scalar.activation(
                                e[:, i, :], sp[:, i, :], AF.Exp,
                                bias=ub[:, mj:mj + 1], scale=1.0 / ALPHA_K)
                    else:
                        nc.scalar.activation(e, sp, AF.Exp, bias=negc,
                                             scale=1.0 / ALPHA_K)
                    pend[g] = e
                if g >= LAG:
                    s = g - LAG
                    # Stagger: emit no consumers at each tile's first step and
                    # two pairs at its second, so the PE-queue wait for the
                    # previous tile's yh-bank release hides behind an extra
                    # score step instead of starving the scalar engine.
                    if s % NPAIR == 0:
                        cons_pairs = []
                    elif s % NPAIR == 1:
                        cons_pairs = [s - 1, s]
                    else:
                        cons_pairs = [s]
                    for cs in cons_pairs:
                        lt, j = divmod(cs, NPAIR)
                        if j == 0:
                            yhp = ps_yh.tile([P, CO, LB], F32, tag="yh",
                                             name=f"yh{lt}")
                            zp = ps_z.tile([32, LB], F32, tag="z",
                                           name=f"z{lt}")
                            state[lt] = (yhp, zp)
                        yhp, zp = state.pop(lt) if j == NPAIR - 1 \
                            else state[lt]
                        e = pend.pop(cs)
                        st, fin = (j == 0), (j == NPAIR - 1)
                        for oc in range(CO):
                            nc.tensor.matmul(
                                yhp[:, oc, :],
                                vt8[:, 2 * j:2 * j + 2, oc * P:(oc + 1) * P],
                                e, start=st, stop=fin, perf_mode=DR)
                        nc.tensor.matmul(zp, ones_pair_e5, e,
                                         start=st, stop=fin, perf_mode=DR)
                    if cons_pairs and fin:
                        # epilogue phase 1 on the DVE.  Normally the yhp copy
                        # goes first (frees the single yh PSUM buffer for the
                        # next tile); for the LAST tile the reciprocal chain
                        # goes first instead — it is the critical path of the
                        # kernel tail and there is no next tile to unblock.
                        u = t_pool.tile([P, CO, LB], F32, tag="t",
                                        name=f"u{lt}")
                        rz = rz_pool.tile([1, LB], F32, tag="rz",
                                          name=f"rz{lt}")
                        rzb = rz_pool.tile([1, LB], BF16, tag="rzb",
                                           name=f"rzb{lt}")
                        if lt == NB - 1:
                            nc.vector.reciprocal_approx_fast(out=rz,
                                                             in_=zp[0:1, :])
                            nc.vector.tensor_copy(rzb, rz)
                            nc.vector.tensor_copy(u, yhp)
                        else:
                            nc.vector.tensor_copy(u, yhp)
                            nc.vector.reciprocal_approx_fast(out=rz,
                                                             in_=zp[0:1, :])
                            nc.vector.tensor_copy(rzb, rz)
                        pend_ph2 = make_phase2(lt, u, rzb, None)
                        pend_ph2_g = g
                # flush a deferred phase 2 one step after it was scheduled
                if pend_ph2 is not None and g > pend_ph2_g:
                    pend_ph2()
                    pend_ph2 = None
            if pend_ph2 is not None:
                pend_ph2()

    nc.compile()
    return nc


def _get_nc(with_ubias=False, with_bv=False):
    key = (bool(with_ubias), bool(with_bv))
    if key not in _CACHE:
        _CACHE[key] = _build(*key)
    return _CACHE[key]


def _to_ci_cc(a, n):
    """[C, n] -> [ci, cc, n] with c = cc*128 + ci."""
    return np.ascontiguousarray(a.reshape(CO, P, n).transpose(1, 0, 2))


def kernel(x, Wq, bq, Wk, bk, Wv, bv, attn_gate, _run_kwargs=None):
    x = np.asarray(x, dtype=np.float32)
    Wq = np.asarray(Wq, dtype=np.float32)
    Wk = np.asarray(Wk, dtype=np.float32)
    Wv = np.asarray(Wv, dtype=np.float32)
    bq = np.asarray(bq, dtype=np.float32)
    bv = np.asarray(bv, dtype=np.float32)
    gate = float(np.tanh(np.asarray(attn_gate, dtype=np.float64))[0])

    sS = np.float32(INV_SQRT2) / np.sqrt(np.float32(C))
    H = (Wk.T @ Wq).astype(np.float32) * sS                 # [c, o]
    wh8 = _to_ci_cc((H * np.float32(BETA_H)), C).astype(NE4)
    wv8 = _to_ci_cc(Wv.T * np.float32(BETA_V * INV_SQRT2 * gate), C).astype(NE4)
    bvb = (bv * np.float32(BETA_V * gate)).astype(NBF)[None, :]

    use_ub = bool(np.any(bq))
    use_bv = bool(np.any(bv))
    nc = _get_nc(use_ub, use_bv)

    in_maps = []
    for b in range(B):
        xc = x[b]
        xh = xc[:, 0::2] - xc[:, 1::2]                       # raw haar band
        xcc = _to_ci_cc(xc, L)
        m = {
            "xb": xcc.astype(NBF),
            "x8": xcc.astype(NE4),
            "xh8": _to_ci_cc(xh, M).astype(NE4),
            "wh8": wh8, "wv8": wv8,
        }
        if use_bv:
            m["bvb"] = bvb
        if use_ub:
            u = (xh.T @ (Wk.T @ bq)) * sS                     # [M]
            m["ub"] = np.ascontiguousarray(
                (u - np.float32(CSH)).astype(np.float32).reshape(MJ, P).T)
        in_maps.append(m)

    res = bass_utils.run_bass_kernel_spmd(
        nc, in_maps, core_ids=list(range(B)), **(_run_kwargs or {}))
    # y comes back as [ci, co, l] bf16; reassemble [C, L] fp32.
    out = np.stack([
        np.asarray(res.results[b]["y"]).transpose(1, 0, 2).reshape(C, L)
        .astype(np.float32)
        for b in range(B)])
    if _run_kwargs:
        kernel.last_results = res
    return out


# revision 53
# speedup vs baseline: 1.0308x; 1.0308x over previous
"""Trainium2 Bass kernel for nn_HFGA_54606214201918.

Computation (per batch element b, C=256 channels, L=4096 positions):
    xh  = (x[:, 0::2] - x[:, 1::2]) / sqrt(2)          # Haar high band  [C, L/2]
    q   = Wq @ x + bq                                  # [C, L]
    k   = Wk @ xh + bk                                 # [C, L/2]
    v   = Wv @ xh + bv                                 # [C, L/2]
    attn = softmax_over_keys((k^T q) / sqrt(C))        # [L/2, L]
    out = (v @ attn) * tanh(gate) + x

Sharding: data-parallel over batch B=8 across the 8 NeuronCores (one batch
element per core); weights broadcast. No collectives.

Key optimizations vs the bf16 baseline:
  - Q projection folded away on host: scores S = xh^T H x with
    H = Wk^T Wq * invsqrt2/sqrt(C).  bk never affects the output (per-query
    shift, softmax-invariant); bq only adds a per-KEY shift u[m] which is
    folded into the exp's per-partition bias (separately compiled variant,
    only used when bq != 0).
  - All heavy matmuls run in fp8 DoubleRow perf mode: one instruction
    contracts TWO K=128 tiles at 0.5 cycles per output column (4x bf16).
    Scores use e4m3 operands (x, k' prescaled to ~unit std on host / at the
    PSUM drain); exp outputs and the v/ones operands use e5m2 (no overflow:
    fp8 casts do NOT saturate on TRN2, they go to inf).
  - Softmax shift: E = exp(S - 3) (softmax-invariant, keeps E in e5m2
    comfortable range); denominator Z via a ones-pair DoubleRow matmul;
    normalization applied to the small [C, LB] output via
    reciprocal_approx_fast + a K=1 broadcast matmul, fused with the
    residual add on the vector + gpsimd engines.
  - exp drains TWO PSUM banks per scalar-engine instruction ([128, 2, 512])
    to halve per-instruction overhead; the scalar engine is the bottleneck.
"""
import sys

if '/opt/trn_rl_repo' not in sys.path:
    sys.path.insert(0, '/opt/trn_rl_repo')

import numpy as np
import ml_dtypes

import concourse.bass as bass
import concourse.tile as tile
from concourse import bacc, mybir
from concourse import bass_utils

B, C, L = 8, 256, 4096
M = L // 2            # 2048 keys
P = 128               # partitions
CO = C // P           # 2 channel chunks
LB = 512              # l-tile (one PSUM bank of fp32)
NB = L // LB          # 8 l-tiles
MJ = M // P           # 16 key chunks
NPAIR = MJ // 2       # 8 key-chunk pairs
INV_SQRT2 = 0.7071067811865476

F32 = mybir.dt.float32
F32R = mybir.dt.float32r
BF16 = mybir.dt.bfloat16
E4 = mybir.dt.float8e4
E5 = mybir.dt.float8e5
AF = mybir.ActivationFunctionType
DR = mybir.MatmulPerfMode.DoubleRow

ALPHA_K = 16.0        # stored k' = ALPHA_K * k'_true (so scores PSUM = 16*S)
BETA_H = 256.0        # host prescale of H so fp8 weights are ~unit std
BETA_V = 32.0         # host prescale of Wv^T
CSH = 3.0             # softmax shift: E = exp(S - CSH)

NE4 = ml_dtypes.float8_e4m3   # matches TRN FP8_EXP4 (max 240, has inf)
NBF = ml_dtypes.bfloat16

_CACHE = {}


def _build(with_ubias=False, with_bv=False):
    nc = bacc.Bacc("TRN2", target_bir_lowering=False, debug=False, num_devices=8)

    xb_d = nc.dram_tensor("xb", [P, CO, L], BF16, kind="ExternalInput").ap()
    x8_d = nc.dram_tensor("x8", [P, CO, L], E4, kind="ExternalInput").ap()
    xh8_d = nc.dram_tensor("xh8", [P, CO, M], E4, kind="ExternalInput").ap()
    wh8_d = nc.dram_tensor("wh8", [P, CO, C], E4, kind="ExternalInput").ap()
    wv8_d = nc.dram_tensor("wv8", [P, CO, C], E4, kind="ExternalInput").ap()
    bvb_d = nc.dram_tensor("bvb", [1, C], BF16, kind="ExternalInput").ap() \
        if with_bv else None
    if with_ubias:
        ub_d = nc.dram_tensor("ub", [P, MJ], F32, kind="ExternalInput").ap()
    y_d = nc.dram_tensor("y", [P, CO, L], BF16, kind="ExternalOutput").ap()

    with tile.TileContext(nc) as tc:
        with tc.tile_pool(name="consts", bufs=1) as consts, \
             tc.tile_pool(name="big", bufs=1) as big, \
             tc.tile_pool(name="e", bufs=8) as e_pool, \
             tc.tile_pool(name="rzp", bufs=2) as rz_pool, \
             tc.tile_pool(name="tp", bufs=2) as t_pool, \
             tc.tile_pool(name="op", bufs=2) as out_pool, \
             tc.tile_pool(name="pssp", bufs=2, space="PSUM") as ps_sp, \
             tc.tile_pool(name="psyh", bufs=1, space="PSUM") as ps_yh, \
             tc.tile_pool(name="psz", bufs=1, space="PSUM") as ps_z, \
             tc.tile_pool(name="psbp", bufs=1, space="PSUM") as ps_bp:

            # ---- constants ----
            wh8 = consts.tile([P, CO, C], E4)
            wv8 = consts.tile([P, CO, C], E4)
            bvb = consts.tile([1, C], BF16, name="bvb") if with_bv else None
            ones_row_bf = consts.tile([1, P], BF16)
            ones_pair_e5 = consts.tile([P, CO, 32], E5)
            ones_row_fr = consts.tile([1, P], F32)
            negc = consts.tile([P, 1], F32)

            # ---- big persistent tensors ----
            x8 = big.tile([P, CO, L], E4)          # scores rhs
            xh8 = big.tile([P, CO, M], E4)         # haar band (raw, no 1/sqrt2)
            xb = big.tile([P, CO, L], BF16)        # residual
            k8 = big.tile([P, CO, M], E4)          # 16 * k'_true, [c, m]
            vt8 = big.tile([P, MJ, C], E5)         # gate*(v+bv), [m, c] chunks
            if with_ubias:
                ub = consts.tile([P, MJ], F32)

            # ---- DMA loads: critical-path tensors first, spread over three
            # queues.  xh8 (k'/v projections) and x8 (scores rhs) gate the
            # pipeline start; xb (residual, bf16) is only needed by the
            # epilogues and trickles in on the sync/vector queues.
            nc.gpsimd.dma_start(out=x8[:, :, :L // 4], in_=x8_d[:, :, :L // 4])
            nc.sync.dma_start(out=wh8, in_=wh8_d)
            for q in range(2):
                msl = slice(q * (M // 4), (q + 1) * (M // 4))
                nc.sync.dma_start(out=xh8[:, :, msl], in_=xh8_d[:, :, msl])
            for q in range(2, 4):
                msl = slice(q * (M // 4), (q + 1) * (M // 4))
                nc.gpsimd.dma_start(out=xh8[:, :, msl], in_=xh8_d[:, :, msl])
            nc.sync.dma_start(out=wv8, in_=wv8_d)
            if with_bv:
                nc.sync.dma_start(out=bvb, in_=bvb_d)
            if with_ubias:
                nc.sync.dma_start(out=ub, in_=ub_d)
            for j in range(1, 4):
                sl = slice(j * (L // 4), (j + 1) * (L // 4))
                nc.gpsimd.dma_start(out=x8[:, :, sl], in_=x8_d[:, :, sl])
            for j in range(4):
                sl = slice(j * (L // 4), (j + 1) * (L // 4))
                nc.sync.dma_start(out=xb[:, :, sl], in_=xb_d[:, :, sl])

            nc.vector.memset(ones_row_bf, 1.0)
            nc.vector.memset(ones_pair_e5, 1.0)
            nc.vector.memset(ones_row_fr, 1.0)
            nc.vector.memset(negc, -CSH)

            # PE p-state warm-up: the tensor engine runs at half clock until
            # it has been busy ~3us.  Burn that ramp on dummy broadcasts into
            # the (otherwise still unused) bp bank while the input DMAs are
            # in flight, so the projections and first scores run at full
            # speed.
            warm = consts.tile([1, LB], BF16)
            nc.vector.memset(warm, 0.0)
            for w in range(14):
                wp = ps_bp.tile([P, LB], F32, tag="bp", name=f"warm{w}")
                nc.tensor.matmul(wp, ones_row_bf, warm, start=True, stop=True)

            # ---- projections, k'/v interleaved; k' drains on the (otherwise
            # idle) scalar engine, v drains on the vector engine, so the two
            # chase the PE in parallel instead of serializing on one engine.
            def emit_kproj(b_):
                msl = slice(b_ * LB, (b_ + 1) * LB)
                kp = ps_sp.tile([P, CO, LB], F32, tag="sp", name=f"kp{b_}")
                for oc in range(CO):
                    nc.tensor.matmul(
                        kp[:, oc, :], wh8[:, :, oc * P:(oc + 1) * P],
                        xh8[:, :, msl], start=True, stop=True, perf_mode=DR)
                nc.vector.tensor_scalar_mul(k8[:, :, msl], kp, 1.0 / ALPHA_K)

            def emit_vproj(j):
                vp = ps_sp.tile([P, 2, LB], F32, tag="sp", name=f"vp{j}")
                for i in range(2):
                    mj = 2 * j + i
                    nc.tensor.matmul(
                        vp[:, i, :C], xh8[:, :, mj * P:(mj + 1) * P], wv8,
                        start=True, stop=not with_bv, perf_mode=DR)
                    if with_bv:
                        nc.tensor.matmul(vp[:, i, :C], ones_row_bf, bvb,
                                         start=False, stop=True)
                nc.vector.tensor_scalar_mul(vt8[:, 2 * j:2 * j + 2, :],
                                            vp[:, :, :C], 1.0 / BETA_V)

            # k' bank 0 up front (gates the first scores); the rest of the
            # projections are emitted just-in-time between the first score
            # steps of the global pipeline below.
            emit_kproj(0)
            proj_sched = {
                0: [('k', 1)], 1: [('k', 2)], 2: [('k', 3)],
                3: [('v', 0), ('v', 1)], 4: [('v', 2), ('v', 3)],
                5: [('v', 4), ('v', 5)], 6: [('v', 6), ('v', 7)],
            }

            # ---- attention: ONE global software pipeline over all 64
            # (l-tile, key-pair) steps.  Scores for step g are emitted LAG
            # steps ahead of that step's consumers (yh/z matmuls), ACROSS
            # l-tile boundaries, so the scalar engine (the bottleneck) always
            # has buffered exp work even while an l-tile's epilogue drains.
            # Each epilogue is split: phase 1 (vector ops; the yhp->SBUF copy
            # releases the single yh PSUM buffer fast) runs right after the
            # tile's last consumer; phase 2 (the PE broadcast matmul + the
            # normalize/residual) is deferred one step so the PE queue head
            # never waits on the reciprocal.
            LAG = 5
            TOT = NB * NPAIR
            pend = {}
            state = {}
            pend_ph2 = None
            pend_ph2_g = -1

            def make_phase2(lt, u, rzb, yhp_done):
                sl = slice(lt * LB, (lt + 1) * LB)

                def ph2():
                    bp = ps_bp.tile([P, LB], F32, tag="bp", name=f"bp{lt}")
                    nc.tensor.matmul(bp, ones_row_bf, rzb,
                                     start=True, stop=True)
                    # DVE cannot read two PSUM operands in one op: stage the
                    # broadcast row through SBUF.
                    b_sb = rz_pool.tile([P, LB], F32, tag="bsb",
                                        name=f"bsb{lt}")
                    nc.vector.tensor_copy(b_sb, bp)
                    t = t_pool.tile([P, CO, LB], BF16, tag="t2",
                                    name=f"t{lt}")
                    o = out_pool.tile([P, CO, LB], BF16, tag="o",
                                      name=f"o{lt}")
                    # per-oc, alternating engines, store halves as they land:
                    # shortens the serial chain that forms the kernel tail
                    # after the last exp.
                    for oc in range(CO):
                        nc.vector.tensor_mul(t[:, oc, :], u[:, oc, :], b_sb)
                        (nc.vector if oc else nc.gpsimd).tensor_add(
                            o[:, oc, :], t[:, oc, :], xb[:, oc, sl])
                        nc.gpsimd.dma_start(out=y_d[:, oc, sl],
                                            in_=o[:, oc, :])
                return ph2

            for g in range(TOT + LAG):
                for kind, idx in proj_sched.pop(g, ()):
                    (emit_kproj if kind == 'k' else emit_vproj)(idx)
                if g < TOT:
                    lt, j = divmod(g, NPAIR)
                    sl = slice(lt * LB, (lt + 1) * LB)
                    sp = ps_sp.tile([P, 2, LB], F32, tag="sp",
                                    name=f"sp{lt}_{j}")
                    for i in range(2):
                        mj = 2 * j + i
                        nc.tensor.matmul(
                            sp[:, i, :], k8[:, :, mj * P:(mj + 1) * P],
                            x8[:, :, sl], start=True, stop=True, perf_mode=DR)
                    e = e_pool.tile([P, 2, LB], E5, tag="e", name=f"e{lt}_{j}")
                    if with_ubias:
                        for i in range(2):
                            mj = 2 * j + i
                            nc.scalar.activation(
                                e[:, i, :], sp[:, i, :], AF.Exp,
                                bias=ub[:, mj:mj + 1], scale=1.0 / ALPHA_K)
                    else:
                        nc.scalar.activation(e, sp, AF.Exp, bias=negc,
                                             scale=1.0 / ALPHA_K)
                    pend[g] = e
                if g >= LAG:
                    s = g - LAG
                    # Stagger: emit no consumers at each tile's first step and
                    # two pairs at its second, so the PE-queue wait for the
                    # previous tile's yh-bank release hides behind an extra
                    # score step instead of starving the scalar engine.
                    if s % NPAIR == 0:
                        cons_pairs = []
                    elif s % NPAIR == 1:
                        cons_pairs = [s - 1, s]
                    else:
                        cons_pairs = [s]
                    for cs in cons_pairs:
                        lt, j = divmod(cs, NPAIR)
                        if j == 0:
                            yhp = ps_yh.tile([P, CO, LB], F32, tag="yh",
                                             name=f"yh{lt}")
                            zp = ps_z.tile([32, LB], F32, tag="z",
                                           name=f"z{lt}")
                            state[lt] = (yhp, zp)
                        yhp, zp = state.pop(lt) if j == NPAIR - 1 \
                            else state[lt]
                        e = pend.pop(cs)
                        st, fin = (j == 0), (j == NPAIR - 1)
                        # z first: its stop lands before yh's, so the tail's
                        # reciprocal chain (which only needs zp) starts first.
                        nc.tensor.matmul(zp, ones_pair_e5, e,
                                         start=st, stop=fin, perf_mode=DR)
                        for oc in range(CO):
                            nc.tensor.matmul(
                                yhp[:, oc, :],
                                vt8[:, 2 * j:2 * j + 2, oc * P:(oc + 1) * P],
                                e, start=st, stop=fin, perf_mode=DR)
                    if cons_pairs and fin:
                        # epilogue phase 1 on the DVE.  Normally the yhp copy
                        # goes first (frees the single yh PSUM buffer for the
                        # next tile); for the LAST tile the reciprocal chain
                        # goes first instead — it is the critical path of the
                        # kernel tail and there is no next tile to unblock.
                        u = t_pool.tile([P, CO, LB], F32, tag="t",
                                        name=f"u{lt}")
                        rz = rz_pool.tile([1, LB], F32, tag="rz",
                                          name=f"rz{lt}")
                        rzb = rz_pool.tile([1, LB], BF16, tag="rzb",
                                           name=f"rzb{lt}")
                        if lt == NB - 1:
                            nc.vector.reciprocal_approx_fast(out=rz,
                                                             in_=zp[0:1, :])
                            nc.vector.tensor_copy(rzb, rz)
                            nc.vector.tensor_copy(u, yhp)
                        else:
                            nc.vector.tensor_copy(u, yhp)
                            nc.vector.reciprocal_approx_fast(out=rz,
                                                             in_=zp[0:1, :])
                            nc.vector.tensor_copy(rzb, rz)
                        pend_ph2 = make_phase2(lt, u, rzb, None)
                        pend_ph2_g = g
                # flush a deferred phase 2 one step after it was scheduled
                if pend_ph2 is not None and g > pend_ph2_g:
                    pend_ph2()
                    pend_ph2 = None
            if pend_ph2 is not None:
                pend_ph2()

    nc.compile()
    return nc


def _get_nc(with_ubias=False, with_bv=False):
    key = (bool(with_ubias), bool(with_bv))
    if key not in _CACHE:
        _CACHE[key] = _build(*key)
    return _CACHE[key]


def _to_ci_cc(a, n):
    """[C, n] -> [ci, cc, n] with c = cc*128 + ci."""
    return np.ascontiguousarray(a.reshape(CO, P, n).transpose(1, 0, 2))


def kernel(x, Wq, bq, Wk, bk, Wv, bv, attn_gate, _run_kwargs=None):
    x = np.asarray(x, dtype=np.float32)
    Wq = np.asarray(Wq, dtype=np.float32)
    Wk = np.asarray(Wk, dtype=np.float32)
    Wv = np.asarray(Wv, dtype=np.float32)
    bq = np.asarray(bq, dtype=np.float32)
    bv = np.asarray(bv, dtype=np.float32)
    gate = float(np.tanh(np.asarray(attn_gate, dtype=np.float64))[0])

    sS = np.float32(INV_SQRT2) / np.sqrt(np.float32(C))
    H = (Wk.T @ Wq).astype(np.float32) * sS                 # [c, o]
    wh8 = _to_ci_cc((H * np.float32(BETA_H)), C).astype(NE4)
    wv8 = _to_ci_cc(Wv.T * np.float32(BETA_V * INV_SQRT2 * gate), C).astype(NE4)
    bvb = (bv * np.float32(BETA_V * gate)).astype(NBF)[None, :]

    use_ub = bool(np.any(bq))
    use_bv = bool(np.any(bv))
    nc = _get_nc(use_ub, use_bv)

    in_maps = []
    for b in range(B):
        xc = x[b]
        xh = xc[:, 0::2] - xc[:, 1::2]                       # raw haar band
        xcc = _to_ci_cc(xc, L)
        m = {
            "xb": xcc.astype(NBF),
            "x8": xcc.astype(NE4),
            "xh8": _to_ci_cc(xh, M).astype(NE4),
            "wh8": wh8, "wv8": wv8,
        }
        if use_bv:
            m["bvb"] = bvb
        if use_ub:
            u = (xh.T @ (Wk.T @ bq)) * sS                     # [M]
            m["ub"] = np.ascontiguousarray(
                (u - np.float32(CSH)).astype(np.float32).reshape(MJ, P).T)
        in_maps.append(m)

    res = bass_utils.run_bass_kernel_spmd(
        nc, in_maps, core_ids=list(range(B)), **(_run_kwargs or {}))
    # y comes back as [ci, co, l] bf16; reassemble [C, L] fp32.
    out = np.stack([
        np.asarray(res.results[b]["y"]).transpose(1, 0, 2).reshape(C, L)
        .astype(np.float32)
        for b in range(B)])
    if _run_kwargs:
        kernel.last_results = res
    return out


# revision 56
# speedup vs baseline: 1.0657x; 1.0338x over previous
"""Trainium2 Bass kernel for nn_HFGA_54606214201918.

Computation (per batch element b, C=256 channels, L=4096 positions):
    xh  = (x[:, 0::2] - x[:, 1::2]) / sqrt(2)          # Haar high band  [C, L/2]
    q   = Wq @ x + bq                                  # [C, L]
    k   = Wk @ xh + bk                                 # [C, L/2]
    v   = Wv @ xh + bv                                 # [C, L/2]
    attn = softmax_over_keys((k^T q) / sqrt(C))        # [L/2, L]
    out = (v @ attn) * tanh(gate) + x

Sharding: data-parallel over batch B=8 across the 8 NeuronCores (one batch
element per core); weights broadcast. No collectives.

Key optimizations vs the bf16 baseline:
  - Q projection folded away on host: scores S = xh^T H x with
    H = Wk^T Wq * invsqrt2/sqrt(C).  bk never affects the output (per-query
    shift, softmax-invariant); bq only adds a per-KEY shift u[m] which is
    folded into the exp's per-partition bias (separately compiled variant,
    only used when bq != 0).
  - All heavy matmuls run in fp8 DoubleRow perf mode: one instruction
    contracts TWO K=128 tiles at 0.5 cycles per output column (4x bf16).
    Scores use e4m3 operands (x, k' prescaled to ~unit std on host / at the
    PSUM drain); exp outputs and the v/ones operands use e5m2 (no overflow:
    fp8 casts do NOT saturate on TRN2, they go to inf).
  - Softmax shift: E = exp(S - 3) (softmax-invariant, keeps E in e5m2
    comfortable range); denominator Z via a ones-pair DoubleRow matmul;
    normalization applied to the small [C, LB] output via
    reciprocal_approx_fast + a K=1 broadcast matmul, fused with the
    residual add on the vector + gpsimd engines.
  - exp drains TWO PSUM banks per scalar-engine instruction ([128, 2, 512])
    to halve per-instruction overhead; the scalar engine is the bottleneck.
"""
import sys

if '/opt/trn_rl_repo' not in sys.path:
    sys.path.insert(0, '/opt/trn_rl_repo')

import numpy as np
import ml_dtypes

import concourse.bass as bass
import concourse.tile as tile
from concourse import bacc, mybir
from concourse import bass_utils

B, C, L = 8, 256, 4096
M = L // 2            # 2048 keys
P = 128               # partitions
CO = C // P           # 2 channel chunks
LB = 512              # l-tile (one PSUM bank of fp32)
NB = L // LB          # 8 l-tiles
MJ = M // P           # 16 key chunks
NPAIR = MJ // 2       # 8 key-chunk pairs
INV_SQRT2 = 0.7071067811865476

F32 = mybir.dt.float32
F32R = mybir.dt.float32r
BF16 = mybir.dt.bfloat16
E4 = mybir.dt.float8e4
E5 = mybir.dt.float8e5
AF = mybir.ActivationFunctionType
DR = mybir.MatmulPerfMode.DoubleRow

ALPHA_K = 16.0        # stored k' = ALPHA_K * k'_true (so scores PSUM = 16*S)
BETA_H = 256.0        # host prescale of H so fp8 weights are ~unit std
BETA_V = 32.0         # host prescale of Wv^T
CSH = 3.0             # softmax shift: E = exp(S - CSH)

NE4 = ml_dtypes.float8_e4m3   # matches TRN FP8_EXP4 (max 240, has inf)
NBF = ml_dtypes.bfloat16

_CACHE = {}


def _build(with_ubias=False, with_bv=False):
    nc = bacc.Bacc("TRN2", target_bir_lowering=False, debug=False, num_devices=8)

    xb_d = nc.dram_tensor("xb", [P, CO, L], BF16, kind="ExternalInput").ap()
    x8_d = nc.dram_tensor("x8", [P, CO, L], E4, kind="ExternalInput").ap()
    xh8_d = nc.dram_tensor("xh8", [P, CO, M], E4, kind="ExternalInput").ap()
    wh8_d = nc.dram_tensor("wh8", [P, CO, C], E4, kind="ExternalInput").ap()
    wv8_d = nc.dram_tensor("wv8", [P, CO, C], E4, kind="ExternalInput").ap()
    bvb_d = nc.dram_tensor("bvb", [1, C], BF16, kind="ExternalInput").ap() \
        if with_bv else None
    if with_ubias:
        ub_d = nc.dram_tensor("ub", [P, MJ], F32, kind="ExternalInput").ap()
    y_d = nc.dram_tensor("y", [P, CO, L], BF16, kind="ExternalOutput").ap()

    with tile.TileContext(nc) as tc:
        with tc.tile_pool(name="consts", bufs=1) as consts, \
             tc.tile_pool(name="big", bufs=1) as big, \
             tc.tile_pool(name="e", bufs=8) as e_pool, \
             tc.tile_pool(name="rzp", bufs=2) as rz_pool, \
             tc.tile_pool(name="tp", bufs=2) as t_pool, \
             tc.tile_pool(name="op", bufs=2) as out_pool, \
             tc.tile_pool(name="pssp", bufs=2, space="PSUM") as ps_sp, \
             tc.tile_pool(name="psyh", bufs=1, space="PSUM") as ps_yh, \
             tc.tile_pool(name="psz", bufs=1, space="PSUM") as ps_z, \
             tc.tile_pool(name="psbp", bufs=1, space="PSUM") as ps_bp:

            # ---- constants ----
            wh8 = consts.tile([P, CO, C], E4)
            wv8 = consts.tile([P, CO, C], E4)
            bvb = consts.tile([1, C], BF16, name="bvb") if with_bv else None
            ones_row_bf = consts.tile([1, P], BF16)
            ones_pair_e5 = consts.tile([P, CO, 32], E5)
            ones_row_fr = consts.tile([1, P], F32)
            negc = consts.tile([P, 1], F32)

            # ---- big persistent tensors ----
            x8 = big.tile([P, CO, L], E4)          # scores rhs
            xh8 = big.tile([P, CO, M], E4)         # haar band (raw, no 1/sqrt2)
            xb = big.tile([P, CO, L], BF16)        # residual
            k8 = big.tile([P, CO, M], E4)          # 16 * k'_true, [c, m]
            vt8 = big.tile([P, MJ, C], E5)         # gate*(v+bv), [m, c] chunks
            if with_ubias:
                ub = consts.tile([P, MJ], F32)

            # ---- DMA loads: critical-path tensors first, spread over three
            # queues.  xh8 (k'/v projections) and x8 (scores rhs) gate the
            # pipeline start; xb (residual, bf16) is only needed by the
            # epilogues and trickles in on the sync/vector queues.
            # xh8 quarter 0 leads the gpsimd queue: it gates the longest
            # dependency chain (k' proj -> drain -> first scores).  x8q0
            # (scores rhs) follows; xh8 q2/q3 after that, in time for the
            # later k' banks.  wh8 + xh8q1 go on the sync queue in parallel.
            def xh8q(q):
                return slice(q * (M // 4), (q + 1) * (M // 4))

            def x8q(q):
                return slice(q * (L // 4), (q + 1) * (L // 4))

            nc.gpsimd.dma_start(out=xh8[:, :, xh8q(0)], in_=xh8_d[:, :, xh8q(0)])
            nc.sync.dma_start(out=wh8, in_=wh8_d)
            nc.gpsimd.dma_start(out=x8[:, :, x8q(0)], in_=x8_d[:, :, x8q(0)])
            nc.sync.dma_start(out=xh8[:, :, xh8q(1)], in_=xh8_d[:, :, xh8q(1)])
            nc.gpsimd.dma_start(out=xh8[:, :, xh8q(2)], in_=xh8_d[:, :, xh8q(2)])
            nc.gpsimd.dma_start(out=xh8[:, :, xh8q(3)], in_=xh8_d[:, :, xh8q(3)])
            nc.sync.dma_start(out=wv8, in_=wv8_d)
            if with_bv:
                nc.sync.dma_start(out=bvb, in_=bvb_d)
            if with_ubias:
                nc.sync.dma_start(out=ub, in_=ub_d)
            for j in range(1, 4):
                nc.gpsimd.dma_start(out=x8[:, :, x8q(j)], in_=x8_d[:, :, x8q(j)])
            for j in range(4):
                sl = slice(j * (L // 4), (j + 1) * (L // 4))
                nc.sync.dma_start(out=xb[:, :, sl], in_=xb_d[:, :, sl])

            nc.vector.memset(ones_row_bf, 1.0)
            nc.vector.memset(ones_pair_e5, 1.0)
            nc.vector.memset(ones_row_fr, 1.0)
            nc.vector.memset(negc, -CSH)

            # PE p-state warm-up: the tensor engine runs at half clock until
            # it has been busy ~3us.  Burn that ramp on dummy broadcasts into
            # the (otherwise still unused) bp bank while the input DMAs are
            # in flight, so the projections and first scores run at full
            # speed.
            warm = consts.tile([1, LB], BF16)
            wsink = consts.tile([1, 64], F32)
            nc.vector.memset(warm, 0.0)
            for w in range(8):
                wp = ps_bp.tile([P, LB], F32, tag="bp", name=f"warm{w}")
                nc.tensor.matmul(wp, ones_row_bf, warm, start=True, stop=True)
                # tiny reader so dead-store elimination keeps every warm-up
                nc.vector.tensor_copy(wsink[:, 8 * w:8 * w + 8],
                                      wp[0:1, 0:8])

            # ---- projections, k'/v interleaved; k' drains on the (otherwise
            # idle) scalar engine, v drains on the vector engine, so the two
            # chase the PE in parallel instead of serializing on one engine.
            def emit_kproj(b_):
                msl = slice(b_ * LB, (b_ + 1) * LB)
                kp = ps_sp.tile([P, CO, LB], F32, tag="sp", name=f"kp{b_}")
                for oc in range(CO):
                    nc.tensor.matmul(
                        kp[:, oc, :], wh8[:, :, oc * P:(oc + 1) * P],
                        xh8[:, :, msl], start=True, stop=True, perf_mode=DR)
                nc.vector.tensor_scalar_mul(k8[:, :, msl], kp, 1.0 / ALPHA_K)

            def emit_vproj(j):
                vp = ps_sp.tile([P, 2, LB], F32, tag="sp", name=f"vp{j}")
                for i in range(2):
                    mj = 2 * j + i
                    nc.tensor.matmul(
                        vp[:, i, :C], xh8[:, :, mj * P:(mj + 1) * P], wv8,
                        start=True, stop=not with_bv, perf_mode=DR)
                    if with_bv:
                        nc.tensor.matmul(vp[:, i, :C], ones_row_bf, bvb,
                                         start=False, stop=True)
                nc.vector.tensor_scalar_mul(vt8[:, 2 * j:2 * j + 2, :],
                                            vp[:, :, :C], 1.0 / BETA_V)

            # k' bank 0 up front (gates the first scores); the rest of the
            # projections are emitted just-in-time between the first score
            # steps of the global pipeline below.
            emit_kproj(0)
            proj_sched = {
                0: [('k', 1)], 1: [('k', 2)], 2: [('k', 3)],
                3: [('v', 0), ('v', 1)], 4: [('v', 2), ('v', 3)],
                5: [('v', 4), ('v', 5)], 6: [('v', 6), ('v', 7)],
            }

            # ---- attention: ONE global software pipeline over all 64
            # (l-tile, key-pair) steps.  Scores for step g are emitted LAG
            # steps ahead of that step's consumers (yh/z matmuls), ACROSS
            # l-tile boundaries, so the scalar engine (the bottleneck) always
            # has buffered exp work even while an l-tile's epilogue drains.
            # Each epilogue is split: phase 1 (vector ops; the yhp->SBUF copy
            # releases the single yh PSUM buffer fast) runs right after the
            # tile's last consumer; phase 2 (the PE broadcast matmul + the
            # normalize/residual) is deferred one step so the PE queue head
            # never waits on the reciprocal.
            LAG = 5
            TOT = NB * NPAIR
            pend = {}
            state = {}
            pend_ph2 = None
            pend_ph2_g = -1

            def make_phase2(lt, u, rzb, yhp_done):
                sl = slice(lt * LB, (lt + 1) * LB)

                def ph2():
                    bp = ps_bp.tile([P, LB], F32, tag="bp", name=f"bp{lt}")
                    nc.tensor.matmul(bp, ones_row_bf, rzb,
                                     start=True, stop=True)
                    # DVE cannot read two PSUM operands in one op: stage the
                    # broadcast row through SBUF.
                    b_sb = rz_pool.tile([P, LB], F32, tag="bsb",
                                        name=f"bsb{lt}")
                    nc.vector.tensor_copy(b_sb, bp)
                    t = t_pool.tile([P, CO, LB], BF16, tag="t2",
                                    name=f"t{lt}")
                    o = out_pool.tile([P, CO, LB], BF16, tag="o",
                                      name=f"o{lt}")
                    # per-oc, alternating engines, store halves as they land:
                    # shortens the serial chain that forms the kernel tail
                    # after the last exp.
                    for oc in range(CO):
                        nc.vector.tensor_mul(t[:, oc, :], u[:, oc, :], b_sb)
                        (nc.vector if oc else nc.gpsimd).tensor_add(
                            o[:, oc, :], t[:, oc, :], xb[:, oc, sl])
                        nc.gpsimd.dma_start(out=y_d[:, oc, sl],
                                            in_=o[:, oc, :])
                return ph2

            for g in range(TOT + LAG):
                for kind, idx in proj_sched.pop(g, ()):
                    (emit_kproj if kind == 'k' else emit_vproj)(idx)
                if g < TOT:
                    lt, j = divmod(g, NPAIR)
                    sl = slice(lt * LB, (lt + 1) * LB)
                    sp = ps_sp.tile([P, 2, LB], F32, tag="sp",
                                    name=f"sp{lt}_{j}")
                    for i in range(2):
                        mj = 2 * j + i
                        nc.tensor.matmul(
                            sp[:, i, :], k8[:, :, mj * P:(mj + 1) * P],
                            x8[:, :, sl], start=True, stop=True, perf_mode=DR)
                    e = e_pool.tile([P, 2, LB], E5, tag="e", name=f"e{lt}_{j}")
                    if with_ubias:
                        for i in range(2):
                            mj = 2 * j + i
                            nc.scalar.activation(
                                e[:, i, :], sp[:, i, :], AF.Exp,
                                bias=ub[:, mj:mj + 1], scale=1.0 / ALPHA_K)
                    else:
                        nc.scalar.activation(e, sp, AF.Exp, bias=negc,
                                             scale=1.0 / ALPHA_K)
                    pend[g] = e
                if g >= LAG:
                    s = g - LAG
                    # Stagger: emit no consumers at each tile's first step and
                    # two pairs at its second, so the PE-queue wait for the
                    # previous tile's yh-bank release hides behind an extra
                    # score step instead of starving the scalar engine.
                    if s % NPAIR == 0:
                        cons_pairs = []
                    elif s % NPAIR == 1:
                        cons_pairs = [s - 1, s]
                    else:
                        cons_pairs = [s]
                    for cs in cons_pairs:
                        lt, j = divmod(cs, NPAIR)
                        if j == 0:
                            yhp = ps_yh.tile([P, CO, LB], F32, tag="yh",
                                             name=f"yh{lt}")
                            zp = ps_z.tile([32, LB], F32, tag="z",
                                           name=f"z{lt}")
                            state[lt] = (yhp, zp)
                        yhp, zp = state.pop(lt) if j == NPAIR - 1 \
                            else state[lt]
                        e = pend.pop(cs)
                        st, fin = (j == 0), (j == NPAIR - 1)
                        # z first: its stop lands before yh's, so the tail's
                        # reciprocal chain (which only needs zp) starts first.
                        nc.tensor.matmul(zp, ones_pair_e5, e,
                                         start=st, stop=fin, perf_mode=DR)
                        for oc in range(CO):
                            nc.tensor.matmul(
                                yhp[:, oc, :],
                                vt8[:, 2 * j:2 * j + 2, oc * P:(oc + 1) * P],
                                e, start=st, stop=fin, perf_mode=DR)
                    if cons_pairs and fin:
                        # epilogue phase 1 on the DVE.  Normally the yhp copy
                        # goes first (frees the single yh PSUM buffer for the
                        # next tile); for the LAST tile the reciprocal chain
                        # goes first instead — it is the critical path of the
                        # kernel tail and there is no next tile to unblock.
                        u = t_pool.tile([P, CO, LB], F32, tag="t",
                                        name=f"u{lt}")
                        rz = rz_pool.tile([1, LB], F32, tag="rz",
                                          name=f"rz{lt}")
                        rzb = rz_pool.tile([1, LB], BF16, tag="rzb",
                                           name=f"rzb{lt}")
                        if lt == NB - 1:
                            nc.vector.reciprocal_approx_fast(out=rz,
                                                             in_=zp[0:1, :])
                            nc.vector.tensor_copy(rzb, rz)
                            nc.vector.tensor_copy(u, yhp)
                        else:
                            nc.vector.tensor_copy(u, yhp)
                            nc.vector.reciprocal_approx_fast(out=rz,
                                                             in_=zp[0:1, :])
                            nc.vector.tensor_copy(rzb, rz)
                        pend_ph2 = make_phase2(lt, u, rzb, None)
                        pend_ph2_g = g
                # flush a deferred phase 2 one step after it was scheduled
                if pend_ph2 is not None and g > pend_ph2_g:
                    pend_ph2()
                    pend_ph2 = None
            if pend_ph2 is not None:
                pend_ph2()

    nc.compile()
    return nc


def _get_nc(with_ubias=False, with_bv=False):
    key = (bool(with_ubias), bool(with_bv))
    if key not in _CACHE:
        _CACHE[key] = _build(*key)
    return _CACHE[key]


def _to_ci_cc(a, n):
    """[C, n] -> [ci, cc, n] with c = cc*128 + ci."""
    return np.ascontiguousarray(a.reshape(CO, P, n).transpose(1, 0, 2))


def kernel(x, Wq, bq, Wk, bk, Wv, bv, attn_gate, _run_kwargs=None):
    x = np.asarray(x, dtype=np.float32)
    Wq = np.asarray(Wq, dtype=np.float32)
    Wk = np.asarray(Wk, dtype=np.float32)
    Wv = np.asarray(Wv, dtype=np.float32)
    bq = np.asarray(bq, dtype=np.float32)
    bv = np.asarray(bv, dtype=np.float32)
    gate = float(np.tanh(np.asarray(attn_gate, dtype=np.float64))[0])

    sS = np.float32(INV_SQRT2) / np.sqrt(np.float32(C))
    H = (Wk.T @ Wq).astype(np.float32) * sS                 # [c, o]
    wh8 = _to_ci_cc((H * np.float32(BETA_H)), C).astype(NE4)
    wv8 = _to_ci_cc(Wv.T * np.float32(BETA_V * INV_SQRT2 * gate), C).astype(NE4)
    bvb = (bv * np.float32(BETA_V * gate)).astype(NBF)[None, :]

    use_ub = bool(np.any(bq))
    use_bv = bool(np.any(bv))
    nc = _get_nc(use_ub, use_bv)

    in_maps = []
    for b in range(B):
        xc = x[b]
        xh = xc[:, 0::2] - xc[:, 1::2]                       # raw haar band
        xcc = _to_ci_cc(xc, L)
        m = {
            "xb": xcc.astype(NBF),
            "x8": xcc.astype(NE4),
            "xh8": _to_ci_cc(xh, M).astype(NE4),
            "wh8": wh8, "wv8": wv8,
        }
        if use_bv:
            m["bvb"] = bvb
        if use_ub:
            u = (xh.T @ (Wk.T @ bq)) * sS                     # [M]
            m["ub"] = np.ascontiguousarray(
                (u - np.float32(CSH)).astype(np.float32).reshape(MJ, P).T)
        in_maps.append(m)

    res = bass_utils.run_bass_kernel_spmd(
        nc, in_maps, core_ids=list(range(B)), **(_run_kwargs or {}))
    # y comes back as [ci, co, l] bf16; reassemble [C, L] fp32.
    out = np.stack([
        np.asarray(res.results[b]["y"]).transpose(1, 0, 2).reshape(C, L)
        .astype(np.float32)
        for b in range(B)])
    if _run_kwargs:
        kernel.last_results = res
    return out
